# revision 1
# baseline (speedup 1.0000x reference)
"""Single-head causal attention (B=16, T=2048, C=1024, D=128) on 8 TRN2 cores.

Data-parallel over batch: each core handles 2 batches. Inside each core:
  xT = transpose(x) via PE transposes           [C on partitions]
  qT/kT/vT = W.T @ xT  (fp32r matmuls)          [D on partitions, T free]
  V = transpose(vT)                             [Tk on partitions, D free]
  per 512-wide query block, per 128-key tile:
    S^T tile = kT_tile.T @ qT_block             [Tk part, Tq free] (PSUM)
    + causal mask on diagonal tiles (DVE add)
    P^T = exp(scale * S^T)  (ACT, writes f32r SBUF)
    OT  += V_tile.T @ P^T                       [D part, Tq free]
    rsum += ones.T @ P^T                        [1, Tq]
  rsumT = tiny transpose matmuls -> [Tq part, 1] ; recip on DVE
  O = transpose(OT) normalized during PSUM evac by recipT (tensor_scalar_mul)
"""

import numpy as np

import concourse.bacc as bacc
import concourse.mybir as mybir
import concourse.tile as tile
from concourse.bass_utils import run_bass_kernel_spmd
from concourse.masks import make_identity

F32 = mybir.dt.float32
F32R = mybir.dt.float32r

B, T, C, D = 16, 2048, 1024, 128
NCORES = 8
BLOC = B // NCORES  # batches per core
NBLK = T // 512  # query blocks of width 512
NKT = T // 128  # key tiles of 128
SCALE = float(D) ** -0.5


def _build():
    nc = bacc.Bacc("TRN2", target_bir_lowering=False, debug=False, num_devices=NCORES)
    x_d = nc.dram_tensor("x", [BLOC, T, C], F32, kind="ExternalInput").ap()
    wq_d = nc.dram_tensor("Wq", [C, D], F32, kind="ExternalInput").ap()
    wk_d = nc.dram_tensor("Wk", [C, D], F32, kind="ExternalInput").ap()
    wv_d = nc.dram_tensor("Wv", [C, D], F32, kind="ExternalInput").ap()
    out_d = nc.dram_tensor("out", [BLOC, T, D], F32, kind="ExternalOutput").ap()

    with tile.TileContext(nc) as tc:
        _emit(nc, tc, x_d, (wq_d, wk_d, wv_d), out_d)
    nc.compile()
    return nc


def _emit(nc, tc, x_d, w_ds, out_d):
    from contextlib import ExitStack

    ctx = ExitStack()
    with ctx:
        const = ctx.enter_context(tc.tile_pool(name="const", bufs=1))
        xtp = ctx.enter_context(tc.tile_pool(name="xtp", bufs=1))
        stage = ctx.enter_context(tc.tile_pool(name="stage", bufs=3))
        qkv = ctx.enter_context(tc.tile_pool(name="qkv", bufs=2))
        ppool = ctx.enter_context(tc.tile_pool(name="ppool", bufs=12))
        small = ctx.enter_context(tc.tile_pool(name="small", bufs=3))
        ps_t = ctx.enter_context(tc.tile_pool(name="ps_t", bufs=1, space="PSUM"))
        ps_proj = ctx.enter_context(tc.tile_pool(name="ps_proj", bufs=2, space="PSUM"))
        ps_s = ctx.enter_context(tc.tile_pool(name="ps_s", bufs=3, space="PSUM"))
        ps_ot = ctx.enter_context(tc.tile_pool(name="ps_ot", bufs=1, space="PSUM"))
        ps_rs = ctx.enter_context(tc.tile_pool(name="ps_rs", bufs=1, space="PSUM"))

        # ---- constants ----
        ident = const.tile([128, 128], F32, tag="ident")
        make_identity(nc, ident)
        masks = const.tile([128, 4, 512], F32, tag="masks")
        nc.gpsimd.memset(masks, 0.0)
        for i in range(4):
            # valid (keep 0.0) iff q - k - 128*i >= 0 ; else fill -1e30
            nc.gpsimd.affine_select(
                out=masks[:, i, :],
                in_=masks[:, i, :],
                compare_op=mybir.AluOpType.is_ge,
                fill=-1e30,
                base=-128 * i,
                pattern=[[1, 512]],
                channel_multiplier=-1,
            )
        ones_f32 = const.tile([128, 1], F32, tag="ones_f32")
        nc.vector.memset(ones_f32, 1.0)
        ones_col = const.tile([128, 1], F32R, tag="ones")
        nc.vector.tensor_copy(ones_col, ones_f32)
        one_one = const.tile([1, 1], F32, tag="oneone")
        nc.vector.memset(one_one, 1.0)
        w_t = const.tile([128, 3, 8, 128], F32R, tag="w")
        for p in range(3):
            nc.sync.dma_start(
                out=w_t[:, p],
                in_=w_ds[p].bitcast(F32R).rearrange("(k p) d -> p k d", p=128),
            )

        # evac engine round-robin (PSUM -> SBUF copies)
        evac_state = [0]

        def evac(out_ap, in_ap):
            if evac_state[0] % 2 == 0:
                nc.vector.tensor_copy(out_ap, in_ap)
            else:
                nc.scalar.copy(out_ap, in_ap)
            evac_state[0] += 1

        for b in range(BLOC):
            # ---- phase X: load x and transpose to xT [C-part, T] ----
            xT = xtp.tile([128, 8, T], F32R, tag="xT")
            for g in range(T // 512):
                for cc in range(8):
                    st = stage.tile([128, 4, 128], F32, tag="stage")
                    nc.sync.dma_start(
                        out=st,
                        in_=x_d[
                            b, 512 * g : 512 * (g + 1), 128 * cc : 128 * (cc + 1)
                        ].rearrange("(ts p) c -> p ts c", p=128),
                    )
                    tp = ps_t.tile([128, 512], F32, tag="tpose")
                    for ts in range(4):
                        nc.tensor.transpose(
                            tp[:, 128 * ts : 128 * (ts + 1)], st[:, ts, :], ident
                        )
                    evac(xT[:, cc, 512 * g : 512 * (g + 1)], tp)

            # ---- phase P: projections qT/kT [D-part, T], V [Tk-part, D] ----
            qT = qkv.tile([128, T], F32R, tag="qT")
            kT = qkv.tile([128, T], F32R, tag="kT")
            V = qkv.tile([128, NKT, 128], F32R, tag="V")
            for j in range(NBLK):
                sl = slice(512 * j, 512 * (j + 1))
                for p, dst in ((0, qT), (1, kT), (2, None)):
                    acc = ps_proj.tile([128, 512], F32, tag="proj")
                    for kk in range(8):
                        nc.tensor.matmul(
                            acc,
                            w_t[:, p, kk],
                            xT[:, kk, sl],
                            start=(kk == 0),
                            stop=(kk == 7),
                        )
                    if dst is not None:
                        evac(dst[:, sl], acc)
                    else:
                        vt_tmp = small.tile([128, 512], F32, tag="vt")
                        evac(vt_tmp, acc)
                        vp = ps_t.tile([128, 512], F32, tag="tpose")
                        for m in range(4):
                            nc.tensor.transpose(
                                vp[:, 128 * m : 128 * (m + 1)],
                                vt_tmp[:, 128 * m : 128 * (m + 1)],
                                ident,
                            )
                        evac(V[:, 4 * j : 4 * (j + 1)].rearrange("p m d -> p (m d)"), vp)

            # ---- phase A: attention ----
            for j in range(NBLK):
                sl = slice(512 * j, 512 * (j + 1))
                ntk = 4 * (j + 1)
                ot = ps_ot.tile([128, 512], F32, tag="ot")
                rs = ps_rs.tile([1, 512], F32, tag="rs")
                for tk in range(ntk):
                    sp = ps_s.tile([128, 512], F32, tag="s")
                    nc.tensor.matmul(
                        sp,
                        kT[:, 128 * tk : 128 * (tk + 1)],
                        qT[:, sl],
                        start=True,
                        stop=True,
                    )
                    if tk >= 4 * j:
                        nc.vector.tensor_tensor(
                            sp, sp, masks[:, tk - 4 * j, :], mybir.AluOpType.add
                        )
                    pt = ppool.tile([128, 512], F32R, tag="p")
                    nc.scalar.activation(
                        pt, sp, mybir.ActivationFunctionType.Exp, scale=SCALE
                    )
                    nc.tensor.matmul(
                        ot, V[:, tk], pt, start=(tk == 0), stop=(tk == ntk - 1)
                    )
                    nc.tensor.matmul(
                        rs, ones_col, pt, start=(tk == 0), stop=(tk == ntk - 1)
                    )
                # rowsum -> transposed reciprocal
                rs_sb = small.tile([1, 512], F32, tag="rssb")
                nc.vector.tensor_copy(rs_sb, rs)
                rsT = ps_rs.tile([128, 4], F32, tag="rs")
                for t in range(4):
                    nc.tensor.matmul(
                        rsT[:, t : t + 1],
                        rs_sb[0:1, 128 * t : 128 * (t + 1)],
                        one_one,
                        start=True,
                        stop=True,
                    )
                recipT = small.tile([128, 4], F32, tag="recip")
                nc.vector.reciprocal(recipT, rsT)
                # OT -> SBUF, transpose to natural layout, normalize, DMA out
                ot_sb = small.tile([128, 512], F32, tag="otsb")
                evac(ot_sb, ot)
                op = ps_t.tile([128, 512], F32, tag="tpose")
                for t in range(4):
                    nc.tensor.transpose(
                        op[:, 128 * t : 128 * (t + 1)],
                        ot_sb[:, 128 * t : 128 * (t + 1)],
                        ident,
                    )
                o_sb = small.tile([128, 4, 128], F32, tag="osb")
                for t in range(4):
                    nc.vector.tensor_scalar_mul(
                        o_sb[:, t, :],
                        op[:, 128 * t : 128 * (t + 1)],
                        recipT[:, t : t + 1],
                    )
                nc.sync.dma_start(
                    out=out_d[b, sl, :].rearrange("(t p) d -> p t d", p=128),
                    in_=o_sb,
                )


_NC = None


def _get_nc():
    global _NC
    if _NC is None:
        _NC = _build()
    return _NC


def kernel(x, Wq, Wk, Wv):
    nc = _get_nc()
    x = np.ascontiguousarray(x, dtype=np.float32)
    in_maps = [
        {"x": x[BLOC * c : BLOC * (c + 1)], "Wq": Wq, "Wk": Wk, "Wv": Wv}
        for c in range(NCORES)
    ]
    res = run_bass_kernel_spmd(nc, in_maps, core_ids=list(range(NCORES)))
    return np.concatenate([res.results[c]["out"] for c in range(NCORES)], axis=0)



# revision 3
# speedup vs baseline: 1.4320x; 1.4320x over previous
"""Single-head causal attention (B=16, T=2048, C=1024, D=128) on 8 TRN2 cores.

Data-parallel over batch: each core handles 2 batches. All-bf16 compute
(measured rel err ~0.5% vs the 2e-2 gate; fp8 fails the max-err metric).

Per core, per batch b:
  X(b,g): x rows [512g..512g+512) f32 -> PE transposes (f32r bitcast data,
          bf16 identity: 1 cyc/row) -> evac-convert -> xT bf16 [C-part, T]
  P(b,j): qT/kT = W^T @ xT (512-wide moving), V = x @ Wv via xT-stationary
          chunks [T-part, D]
  A(b,j): per key tile (causally width-restricted to queries >= 128*iloc):
          S^T = kT_tile^T @ qT (PSUM), + triangular -1e30 mask via a PE
          matmul accumulate (ident^T @ tri) on the diagonal square,
          P^T = exp(scale*S^T) on ACT (bf16),
          OT += V_tile^T @ P^T,  rsT[:,ch] += pt_chunk^T @ ones (out [128,1]).
          Normalize via per-partition recip after the out-transpose.

Emission is a single stream with software pipelining: within an A block the
OT/rsT for tile i are emitted two tiles behind its scores, and X/P work for
later groups/batches is popped from a filler queue ahead of each OT so the
in-order PE queue never parks on an exp dependency.
"""

import numpy as np
from collections import deque

import concourse.bacc as bacc
import concourse.mybir as mybir
import concourse.tile as tile
from concourse.bass_utils import run_bass_kernel_spmd
from concourse.masks import make_identity

F32 = mybir.dt.float32
F32R = mybir.dt.float32r
BF16 = mybir.dt.bfloat16

B, T, C, D = 16, 2048, 1024, 128
NCORES = 8
BLOC = B // NCORES  # batches per core
NBLK = T // 512  # query blocks of width 512
NKT = T // 128  # key tiles of 128
SCALE = float(D) ** -0.5


def _build():
    nc = bacc.Bacc("TRN2", target_bir_lowering=False, debug=False, num_devices=NCORES)
    x_d = nc.dram_tensor("x", [BLOC, T, C], F32, kind="ExternalInput").ap()
    wq_d = nc.dram_tensor("Wq", [C, D], F32, kind="ExternalInput").ap()
    wk_d = nc.dram_tensor("Wk", [C, D], F32, kind="ExternalInput").ap()
    wv_d = nc.dram_tensor("Wv", [C, D], F32, kind="ExternalInput").ap()
    out_d = nc.dram_tensor("out", [BLOC, T, D], F32, kind="ExternalOutput").ap()

    with tile.TileContext(nc) as tc:
        _emit(nc, tc, x_d, (wq_d, wk_d, wv_d), out_d)
    nc.compile()
    return nc


def _emit(nc, tc, x_d, w_ds, out_d):
    from contextlib import ExitStack

    ctx = ExitStack()
    with ctx:
        const = ctx.enter_context(tc.tile_pool(name="const", bufs=1))
        xstage = ctx.enter_context(tc.tile_pool(name="xstage", bufs=6))
        xtp = ctx.enter_context(tc.tile_pool(name="xtp", bufs=1))
        qkv = ctx.enter_context(tc.tile_pool(name="qkv", bufs=2))
        ppool = ctx.enter_context(tc.tile_pool(name="ppool", bufs=8))
        small = ctx.enter_context(tc.tile_pool(name="small", bufs=4))
        # PSUM: 8 banks of [128, 2KB]
        ps_t = ctx.enter_context(tc.tile_pool(name="ps_t", bufs=2, space="PSUM"))
        ps_proj = ctx.enter_context(tc.tile_pool(name="ps_proj", bufs=2, space="PSUM"))
        ps_s = ctx.enter_context(tc.tile_pool(name="ps_s", bufs=2, space="PSUM"))
        ps_ot = ctx.enter_context(tc.tile_pool(name="ps_ot", bufs=1, space="PSUM"))
        ps_rs = ctx.enter_context(tc.tile_pool(name="ps_rs", bufs=1, space="PSUM"))

        # ---- x stage DMAs: one 512KB slab per 128 t-rows ----
        stages = {}

        def load_stage(b, s):
            st = xstage.tile([128, C], F32R, tag="stage", name="stage")
            nc.sync.dma_start(
                out=st, in_=x_d[b, 128 * s : 128 * (s + 1), :].bitcast(F32R)
            )
            stages[(b, s)] = st

        for _s in range(4):
            load_stage(0, _s)

        # ---- constants ----
        ident_f = const.tile([128, 128], F32, tag="identf")
        make_identity(nc, ident_f)
        ident = const.tile([128, 128], BF16, tag="ident")
        nc.gpsimd.tensor_copy(ident, ident_f)
        ident_r = const.tile([128, 128], F32R, tag="identr")
        nc.gpsimd.tensor_copy(ident_r, ident_f)
        # triangular mask for the diagonal square: keep 0 iff q - k >= 0
        tri_f = const.tile([128, 128], F32, tag="trif")
        nc.gpsimd.memset(tri_f, 0.0)
        nc.gpsimd.affine_select(
            out=tri_f,
            in_=tri_f,
            compare_op=mybir.AluOpType.is_ge,
            fill=-1e30,
            base=0,
            pattern=[[1, 128]],
            channel_multiplier=-1,
        )
        tri = const.tile([128, 128], BF16, tag="tri")
        nc.gpsimd.tensor_copy(tri, tri_f)
        ones_col = const.tile([128, 1], BF16, tag="ones")
        nc.vector.memset(ones_col, 1.0)
        # weights: fp32 staging -> bf16 [128, 3, 8, 128]
        w_bf = const.tile([128, 3, 8, 128], BF16, tag="wbf")
        for p in range(3):
            w_st = const.tile([128, 8, 128], F32, tag=f"wst{p}")
            nc.sync.dma_start(
                out=w_st, in_=w_ds[p].rearrange("(k p) d -> p k d", p=128)
            )
            nc.gpsimd.tensor_copy(w_bf[:, p], w_st)

        # evac copies (PSUM -> SBUF) rotate over DVE and ACT
        # (GPSIMD/Pool cannot access PSUM at all).
        copy_fns = [
            lambda o, i: nc.vector.tensor_copy(o, i),
            lambda o, i: nc.scalar.copy(o, i),
        ]
        evac_state = [0]

        def evac(out_ap, in_ap, seq=(0, 1)):
            copy_fns[seq[evac_state[0] % len(seq)]](out_ap, in_ap)
            evac_state[0] += 1

        xTs = {}
        qTs, kTs, Vs = {}, {}, {}

        # ---------- emission units ----------
        def xgroup(b, s, half):
            """Transpose half of the [128 t, C] slab s: 4 c-chunks."""
            st = stages[(b, s)]
            xT = xTs[b]
            tp = ps_t.tile([128, 4, 128], F32R, tag="tpose")
            for i in range(4):
                cc = 4 * half + i
                nc.tensor.transpose(
                    tp[:, i],
                    st[:, 128 * cc : 128 * (cc + 1)],
                    ident_r,
                )
            evac(
                xT[:, 4 * half : 4 * half + 4, 128 * s : 128 * (s + 1)],
                tp,
            )

        proj_accs = {}

        def proj_qk_slab(b, j, p, s4):
            """qT/kT for one 128-col t-slice: 8 kk matmuls, 128-wide."""
            xT = xTs[b]
            dst = (qTs if p == 0 else kTs)[b]
            if s4 == 0:
                proj_accs[(b, j, p)] = ps_proj.tile(
                    [128, 512], F32, tag="proj", name="proj"
                )
            acc = proj_accs[(b, j, p)]
            tsl = slice(512 * j + 128 * s4, 512 * j + 128 * (s4 + 1))
            for kk in range(8):
                nc.tensor.matmul(
                    acc[:, 128 * s4 : 128 * (s4 + 1)],
                    w_bf[:, p, kk],
                    xT[:, kk, tsl],
                    start=(s4 == 0 and kk == 0),
                    stop=(s4 == 3 and kk == 7),
                    skip_group_check=True,
                )
            if s4 == 3:
                evac(dst[:, 512 * j : 512 * (j + 1)], acc)
                del proj_accs[(b, j, p)]

        def proj_v_sub(b, j, tc4):
            """One 128-row t-chunk of V (8 kk matmuls); tc4 3 evacuates."""
            xT = xTs[b]
            V = Vs[b]
            if tc4 == 0:
                proj_accs[(b, j, 2)] = ps_proj.tile(
                    [128, 512], F32, tag="proj", name="vacc"
                )
            vacc = proj_accs[(b, j, 2)]
            tsl = slice(512 * j + 128 * tc4, 512 * j + 128 * (tc4 + 1))
            for kk in range(8):
                nc.tensor.matmul(
                    vacc[:, 128 * tc4 : 128 * (tc4 + 1)],
                    xT[:, kk, tsl],
                    w_bf[:, 2, kk],
                    start=(tc4 == 0 and kk == 0),
                    stop=(tc4 == 3 and kk == 7),
                    skip_group_check=True,
                )
            if tc4 == 3:
                evac(
                    V[:, 4 * j : 4 * (j + 1)],
                    vacc.rearrange("p (m d) -> p m d", m=4),
                )
                del proj_accs[(b, j, 2)]

        # filler queue: list of (marker, closure). Marker (b, j) means this
        # unit belongs to X/P of block j of batch b; A(b, j) requires all
        # markers <= (b, j) emitted.
        # filler unit = (marker, est_pe_ns, closure); marker = (b, g, kind):
        # kind 0 units must precede A(b, g) block start (xT slabs + qT);
        # kind 1 units (kT/V of block g) are only needed by tiles >= 4g.
        fillers = deque()
        slabs = [(b, s) for b in range(BLOC) for s in range(4 * NBLK)]
        for b in range(BLOC):
            for g in range(NBLK):
                mk0 = (b, g, 0)
                mk1 = (b, g, 1)
                for s4 in range(4):
                    s = 4 * g + s4
                    # prefetch the slab load 4 slabs ahead of its transposes
                    idx = slabs.index((b, s)) + 4
                    if idx < len(slabs):
                        pb, ps2 = slabs[idx]
                        fillers.append(
                            (mk0, 0, lambda pb=pb, ps2=ps2: load_stage(pb, ps2))
                        )
                    for half in range(2):
                        fillers.append(
                            (mk0, 230, lambda b=b, s=s, h=half: xgroup(b, s, h))
                        )
                    # q and v lag one slab behind the transposes so the
                    # xT evacuations are complete when they issue
                    # (2 live proj accumulators: q + v; k comes after)
                    if s4 >= 1:
                        fillers.append(
                            (mk0, 460,
                             lambda b=b, g=g, s=s4 - 1: proj_qk_slab(b, g, 0, s))
                        )
                        fillers.append(
                            (mk1, 460, lambda b=b, g=g, s=s4 - 1: proj_v_sub(b, g, s))
                        )
                fillers.append(
                    (mk0, 460, lambda b=b, g=g: proj_qk_slab(b, g, 0, 3))
                )
                fillers.append((mk1, 460, lambda b=b, g=g: proj_v_sub(b, g, 3)))
                for s4 in range(4):
                    fillers.append(
                        (mk1, 460, lambda b=b, g=g, s=s4: proj_qk_slab(b, g, 1, s))
                    )

        def drain_until(marker):
            # the queue is not strictly marker-sorted (kind-1 units are
            # interleaved), so filter the whole queue
            rest = []
            while fillers:
                mk, est, fn = fillers.popleft()
                if mk <= marker:
                    fn()
                else:
                    rest.append((mk, est, fn))
            fillers.extend(rest)

        def pop_filler(budget_ns=350, force=False):
            spent = 0
            while fillers and spent < budget_ns:
                if not force and len(fillers) <= 8:
                    return
                _, est, fn = fillers.popleft()
                fn()
                spent += est if est else 0

        for b in range(BLOC):
            xTs[b] = xtp.tile([128, 8, T], BF16, tag="xT", name="xT")
            qTs[b] = qkv.tile([128, T], BF16, tag="qT", name="qT")
            kTs[b] = qkv.tile([128, T], BF16, tag="kT", name="kT")
            Vs[b] = qkv.tile([128, NKT, 128], BF16, tag="V", name="V")

        # ---------- main schedule ----------
        drain_until((0, 0, 1))  # startup: PE is DMA-starved, emit eagerly
        for b in range(BLOC):
            for j in range(NBLK):
                drain_until((b, j, 0))
                qT, kT, V = qTs[b], kTs[b], Vs[b]
                sl = slice(512 * j, 512 * (j + 1))
                ntk = 4 * (j + 1)
                ot = ps_ot.tile([128, 512], F32, tag="ot")
                rsT = ps_rs.tile([128, 4], F32, tag="rs")
                pts = {}

                def stage1(tk):
                    iloc = tk - 4 * j
                    q0 = 128 * iloc if iloc >= 0 else 0
                    sp = ps_s.tile([128, 512], F32, tag="s")
                    nc.tensor.matmul(
                        sp[:, q0:],
                        kT[:, 128 * tk : 128 * (tk + 1)],
                        qT[:, 512 * j + q0 : 512 * (j + 1)],
                        start=True,
                        stop=(iloc < 0),
                        skip_group_check=True,
                    )
                    if iloc >= 0:
                        nc.tensor.matmul(
                            sp[:, q0 : q0 + 128],
                            ident,
                            tri,
                            start=False,
                            stop=True,
                            skip_group_check=True,
                        )
                    pt = ppool.tile([128, 512], BF16, tag="p")
                    nc.scalar.activation(
                        pt[:, q0:],
                        sp[:, q0:],
                        mybir.ActivationFunctionType.Exp,
                        scale=SCALE,
                    )
                    pts[tk] = (pt, q0)

                def stage2(tk):
                    pt, q0 = pts.pop(tk)
                    nc.tensor.matmul(
                        ot[:, q0:],
                        V[:, tk],
                        pt[:, q0:],
                        start=(tk == 0),
                        stop=(tk == ntk - 1),
                        skip_group_check=True,
                    )
                    for ch in range(q0 // 128, 4):
                        nc.tensor.matmul(
                            rsT[:, ch : ch + 1],
                            pt[:, 128 * ch : 128 * (ch + 1)],
                            ones_col,
                            start=(tk == 0 and ch == 0),
                            stop=(tk == ntk - 1 and ch == 3),
                            skip_group_check=True,
                        )

                # depth-2 software pipeline with fillers ahead of each OT
                last = b == BLOC - 1 and j == NBLK - 1
                for tk in range(ntk):
                    drain_until((b, tk // 4, 1))
                    stage1(tk)
                    if tk >= 2:
                        pop_filler(force=last)
                        stage2(tk - 2)
                pop_filler(force=last)
                stage2(ntk - 2)
                pop_filler(force=last)
                stage2(ntk - 1)

                # ---- block tail: normalize + transpose + store ----
                # per-quarter chains so the store DMAs overlap the next block
                recipT = small.tile([128, 4], F32, tag="recip")
                nc.vector.reciprocal(recipT, rsT)
                ot_sb = small.tile([128, 512], BF16, tag="otsb")
                o_sb = small.tile([128, 4, 128], F32, tag="osb")
                for t4 in range(4):
                    evac(
                        ot_sb[:, 128 * t4 : 128 * (t4 + 1)],
                        ot[:, 128 * t4 : 128 * (t4 + 1)],
                    )
                    pop_filler(250)
                    op = ps_t.tile([128, 1, 128], BF16, tag="tpose", name="op")
                    nc.tensor.transpose(
                        op[:, 0],
                        ot_sb[:, 128 * t4 : 128 * (t4 + 1)],
                        ident,
                    )
                    if t4 % 2 == 0:
                        nc.vector.tensor_scalar_mul(
                            o_sb[:, t4], op[:, 0], recipT[:, t4 : t4 + 1]
                        )
                    else:
                        nc.scalar.mul(o_sb[:, t4], op[:, 0], recipT[:, t4 : t4 + 1])
                    if last:
                        # final block: store each quarter as it completes so
                        # the closing DMA chain overlaps the remaining work
                        nc.sync.dma_start(
                            out=out_d[
                                b,
                                512 * j + 128 * t4 : 512 * j + 128 * (t4 + 1),
                                :,
                            ].rearrange("(t p) d -> p t d", p=128),
                            in_=o_sb[:, t4],
                        )
                if not last:
                    nc.sync.dma_start(
                        out=out_d[b, sl, :].rearrange("(t p) d -> p t d", p=128),
                        in_=o_sb,
                    )

        # drain any leftover fillers (shouldn't be any)
        while fillers:
            fillers.popleft()[1]()


_NC = None


def _get_nc():
    global _NC
    if _NC is None:
        _NC = _build()
    return _NC


def kernel(x, Wq, Wk, Wv):
    nc = _get_nc()
    x = np.ascontiguousarray(x, dtype=np.float32)
    in_maps = [
        {"x": x[BLOC * c : BLOC * (c + 1)], "Wq": Wq, "Wk": Wk, "Wv": Wv}
        for c in range(NCORES)
    ]
    res = run_bass_kernel_spmd(nc, in_maps, core_ids=list(range(NCORES)))
    return np.concatenate([res.results[c]["out"] for c in range(NCORES)], axis=0)


# revision 4
# speedup vs baseline: 1.4328x; 1.0006x over previous
"""Single-head causal attention (B=16, T=2048, C=1024, D=128) on 8 TRN2 cores.

Data-parallel over batch: each core handles 2 batches. All-bf16 compute
(measured rel err ~0.5% vs the 2e-2 gate; fp8 fails the max-err metric).

Per core, per batch b:
  X(b,g): x rows [512g..512g+512) f32 -> PE transposes (f32r bitcast data,
          bf16 identity: 1 cyc/row) -> evac-convert -> xT bf16 [C-part, T]
  P(b,j): qT/kT = W^T @ xT (512-wide moving), V = x @ Wv via xT-stationary
          chunks [T-part, D]
  A(b,j): per key tile (causally width-restricted to queries >= 128*iloc):
          S^T = kT_tile^T @ qT (PSUM), + triangular -1e30 mask via a PE
          matmul accumulate (ident^T @ tri) on the diagonal square,
          P^T = exp(scale*S^T) on ACT (bf16),
          OT += V_tile^T @ P^T,  rsT[:,ch] += pt_chunk^T @ ones (out [128,1]).
          Normalize via per-partition recip after the out-transpose.

Emission is a single stream with software pipelining: within an A block the
OT/rsT for tile i are emitted two tiles behind its scores, and X/P work for
later groups/batches is popped from a filler queue ahead of each OT so the
in-order PE queue never parks on an exp dependency.
"""

import numpy as np
from collections import deque

import concourse.bacc as bacc
import concourse.mybir as mybir
import concourse.tile as tile
from concourse.bass_utils import run_bass_kernel_spmd
from concourse.masks import make_identity

F32 = mybir.dt.float32
F32R = mybir.dt.float32r
BF16 = mybir.dt.bfloat16

B, T, C, D = 16, 2048, 1024, 128
NCORES = 8
BLOC = B // NCORES  # batches per core
NBLK = T // 512  # query blocks of width 512
NKT = T // 128  # key tiles of 128
SCALE = float(D) ** -0.5


def _build():
    nc = bacc.Bacc("TRN2", target_bir_lowering=False, debug=False, num_devices=NCORES)
    x_d = nc.dram_tensor("x", [BLOC, T, C], F32, kind="ExternalInput").ap()
    wq_d = nc.dram_tensor("Wq", [C, D], F32, kind="ExternalInput").ap()
    wk_d = nc.dram_tensor("Wk", [C, D], F32, kind="ExternalInput").ap()
    wv_d = nc.dram_tensor("Wv", [C, D], F32, kind="ExternalInput").ap()
    out_d = nc.dram_tensor("out", [BLOC, T, D], F32, kind="ExternalOutput").ap()

    with tile.TileContext(nc) as tc:
        _emit(nc, tc, x_d, (wq_d, wk_d, wv_d), out_d)
    nc.compile()
    return nc


def _emit(nc, tc, x_d, w_ds, out_d):
    from contextlib import ExitStack

    ctx = ExitStack()
    with ctx:
        const = ctx.enter_context(tc.tile_pool(name="const", bufs=1))
        xstage = ctx.enter_context(tc.tile_pool(name="xstage", bufs=6))
        xtp = ctx.enter_context(tc.tile_pool(name="xtp", bufs=1))
        qkv = ctx.enter_context(tc.tile_pool(name="qkv", bufs=2))
        ppool = ctx.enter_context(tc.tile_pool(name="ppool", bufs=8))
        small = ctx.enter_context(tc.tile_pool(name="small", bufs=4))
        # PSUM: 8 banks of [128, 2KB]
        ps_t = ctx.enter_context(tc.tile_pool(name="ps_t", bufs=2, space="PSUM"))
        ps_proj = ctx.enter_context(tc.tile_pool(name="ps_proj", bufs=2, space="PSUM"))
        ps_s = ctx.enter_context(tc.tile_pool(name="ps_s", bufs=2, space="PSUM"))
        ps_ot = ctx.enter_context(tc.tile_pool(name="ps_ot", bufs=1, space="PSUM"))
        ps_rs = ctx.enter_context(tc.tile_pool(name="ps_rs", bufs=1, space="PSUM"))

        # ---- x stage DMAs: one 512KB slab per 128 t-rows ----
        stages = {}

        def load_stage(b, s, split=False):
            st = xstage.tile([128, C], F32R, tag="stage", name="stage")
            if split:
                for h in range(2):
                    nc.sync.dma_start(
                        out=st[:, 512 * h : 512 * (h + 1)],
                        in_=x_d[
                            b, 128 * s : 128 * (s + 1), 512 * h : 512 * (h + 1)
                        ].bitcast(F32R),
                    )
            else:
                nc.sync.dma_start(
                    out=st, in_=x_d[b, 128 * s : 128 * (s + 1), :].bitcast(F32R)
                )
            stages[(b, s)] = st

        load_stage(0, 0, split=True)

        # W DMAs interleaved with the early slab loads: Wq before slab 1 so
        # the first q-projection isn't gated on weights
        w_sts = []
        for p in range(3):
            w_st = const.tile([128, 8, 128], F32, tag=f"wst{p}", name="wst")
            nc.sync.dma_start(
                out=w_st, in_=w_ds[p].rearrange("(k p) d -> p k d", p=128)
            )
            w_sts.append(w_st)
            load_stage(0, p + 1)

        # ---- constants ----
        ident_f = const.tile([128, 128], F32, tag="identf")
        make_identity(nc, ident_f)
        ident = const.tile([128, 128], BF16, tag="ident")
        nc.gpsimd.tensor_copy(ident, ident_f)
        ident_r = const.tile([128, 128], F32R, tag="identr")
        nc.gpsimd.tensor_copy(ident_r, ident_f)
        # triangular mask for the diagonal square: keep 0 iff q - k >= 0
        tri_f = const.tile([128, 128], F32, tag="trif")
        nc.gpsimd.memset(tri_f, 0.0)
        nc.gpsimd.affine_select(
            out=tri_f,
            in_=tri_f,
            compare_op=mybir.AluOpType.is_ge,
            fill=-1e30,
            base=0,
            pattern=[[1, 128]],
            channel_multiplier=-1,
        )
        tri = const.tile([128, 128], BF16, tag="tri")
        nc.gpsimd.tensor_copy(tri, tri_f)
        ones_col = const.tile([128, 1], BF16, tag="ones")
        nc.vector.memset(ones_col, 1.0)
        # weights: fp32 staging -> bf16 [128, 3, 8, 128]
        w_bf = const.tile([128, 3, 8, 128], BF16, tag="wbf")
        for p in range(3):
            nc.gpsimd.tensor_copy(w_bf[:, p], w_sts[p])

        # evac copies (PSUM -> SBUF) rotate over DVE and ACT
        # (GPSIMD/Pool cannot access PSUM at all).
        copy_fns = [
            lambda o, i: nc.vector.tensor_copy(o, i),
            lambda o, i: nc.scalar.copy(o, i),
        ]
        evac_state = [0]

        def evac(out_ap, in_ap, seq=(0, 1)):
            copy_fns[seq[evac_state[0] % len(seq)]](out_ap, in_ap)
            evac_state[0] += 1

        xTs = {}
        qTs, kTs, Vs = {}, {}, {}

        # ---------- emission units ----------
        def xgroup(b, s, half):
            """Transpose half of the [128 t, C] slab s: 4 c-chunks."""
            st = stages[(b, s)]
            xT = xTs[b]
            tp = ps_t.tile([128, 4, 128], F32R, tag="tpose")
            for i in range(4):
                cc = 4 * half + i
                nc.tensor.transpose(
                    tp[:, i],
                    st[:, 128 * cc : 128 * (cc + 1)],
                    ident_r,
                )
            evac(
                xT[:, 4 * half : 4 * half + 4, 128 * s : 128 * (s + 1)],
                tp,
            )

        proj_accs = {}

        def proj_qk_slab(b, j, p, s4):
            """qT/kT for one 128-col t-slice: 8 kk matmuls, 128-wide."""
            xT = xTs[b]
            dst = (qTs if p == 0 else kTs)[b]
            if s4 == 0:
                proj_accs[(b, j, p)] = ps_proj.tile(
                    [128, 512], F32, tag="proj", name="proj"
                )
            acc = proj_accs[(b, j, p)]
            tsl = slice(512 * j + 128 * s4, 512 * j + 128 * (s4 + 1))
            for kk in range(8):
                nc.tensor.matmul(
                    acc[:, 128 * s4 : 128 * (s4 + 1)],
                    w_bf[:, p, kk],
                    xT[:, kk, tsl],
                    start=(s4 == 0 and kk == 0),
                    stop=(s4 == 3 and kk == 7),
                    skip_group_check=True,
                )
            if s4 == 3:
                evac(dst[:, 512 * j : 512 * (j + 1)], acc)
                del proj_accs[(b, j, p)]

        def proj_v_sub(b, j, tc4):
            """One 128-row t-chunk of V (8 kk matmuls); tc4 3 evacuates."""
            xT = xTs[b]
            V = Vs[b]
            if tc4 == 0:
                proj_accs[(b, j, 2)] = ps_proj.tile(
                    [128, 512], F32, tag="proj", name="vacc"
                )
            vacc = proj_accs[(b, j, 2)]
            tsl = slice(512 * j + 128 * tc4, 512 * j + 128 * (tc4 + 1))
            for kk in range(8):
                nc.tensor.matmul(
                    vacc[:, 128 * tc4 : 128 * (tc4 + 1)],
                    xT[:, kk, tsl],
                    w_bf[:, 2, kk],
                    start=(tc4 == 0 and kk == 0),
                    stop=(tc4 == 3 and kk == 7),
                    skip_group_check=True,
                )
            if tc4 == 3:
                evac(
                    V[:, 4 * j : 4 * (j + 1)],
                    vacc.rearrange("p (m d) -> p m d", m=4),
                )
                del proj_accs[(b, j, 2)]

        # filler queue: list of (marker, closure). Marker (b, j) means this
        # unit belongs to X/P of block j of batch b; A(b, j) requires all
        # markers <= (b, j) emitted.
        # filler unit = (marker, est_pe_ns, closure); marker = (b, g, kind):
        # kind 0 units must precede A(b, g) block start (xT slabs + qT);
        # kind 1 units (kT/V of block g) are only needed by tiles >= 4g.
        fillers = deque()
        slabs = [(b, s) for b in range(BLOC) for s in range(4 * NBLK)]
        for b in range(BLOC):
            for g in range(NBLK):
                mk0 = (b, g, 0)
                mk1 = (b, g, 1)
                for s4 in range(4):
                    s = 4 * g + s4
                    # prefetch the slab load 4 slabs ahead of its transposes
                    idx = slabs.index((b, s)) + 4
                    if idx < len(slabs):
                        pb, ps2 = slabs[idx]
                        fillers.append(
                            (mk0, 0, lambda pb=pb, ps2=ps2: load_stage(pb, ps2))
                        )
                    for half in range(2):
                        fillers.append(
                            (mk0, 230, lambda b=b, s=s, h=half: xgroup(b, s, h))
                        )
                    # q and v lag one slab behind the transposes so the
                    # xT evacuations are complete when they issue
                    # (2 live proj accumulators: q + v; k comes after)
                    if s4 >= 1:
                        fillers.append(
                            (mk0, 460,
                             lambda b=b, g=g, s=s4 - 1: proj_qk_slab(b, g, 0, s))
                        )
                        fillers.append(
                            (mk1, 460, lambda b=b, g=g, s=s4 - 1: proj_v_sub(b, g, s))
                        )
                fillers.append(
                    (mk0, 460, lambda b=b, g=g: proj_qk_slab(b, g, 0, 3))
                )
                fillers.append((mk1, 460, lambda b=b, g=g: proj_v_sub(b, g, 3)))
                for s4 in range(4):
                    fillers.append(
                        (mk1, 460, lambda b=b, g=g, s=s4: proj_qk_slab(b, g, 1, s))
                    )

        def drain_until(marker):
            # the queue is not strictly marker-sorted (kind-1 units are
            # interleaved), so filter the whole queue
            rest = []
            while fillers:
                mk, est, fn = fillers.popleft()
                if mk <= marker:
                    fn()
                else:
                    rest.append((mk, est, fn))
            fillers.extend(rest)

        def pop_filler(budget_ns=350, force=False):
            spent = 0
            while fillers and spent < budget_ns:
                if not force and len(fillers) <= 8:
                    return
                _, est, fn = fillers.popleft()
                fn()
                spent += est if est else 0

        for b in range(BLOC):
            xTs[b] = xtp.tile([128, 8, T], BF16, tag="xT", name="xT")
            qTs[b] = qkv.tile([128, T], BF16, tag="qT", name="qT")
            kTs[b] = qkv.tile([128, T], BF16, tag="kT", name="kT")
            Vs[b] = qkv.tile([128, NKT, 128], BF16, tag="V", name="V")

        # ---------- main schedule ----------
        drain_until((0, 0, 1))  # startup: PE is DMA-starved, emit eagerly
        for b in range(BLOC):
            for j in range(NBLK):
                drain_until((b, j, 0))
                qT, kT, V = qTs[b], kTs[b], Vs[b]
                sl = slice(512 * j, 512 * (j + 1))
                ntk = 4 * (j + 1)
                ot = ps_ot.tile([128, 512], F32, tag="ot")
                rsT = ps_rs.tile([128, 4], F32, tag="rs")
                pts = {}

                def stage1(tk):
                    iloc = tk - 4 * j
                    q0 = 128 * iloc if iloc >= 0 else 0
                    sp = ps_s.tile([128, 512], F32, tag="s")
                    nc.tensor.matmul(
                        sp[:, q0:],
                        kT[:, 128 * tk : 128 * (tk + 1)],
                        qT[:, 512 * j + q0 : 512 * (j + 1)],
                        start=True,
                        stop=(iloc < 0),
                        skip_group_check=True,
                    )
                    if iloc >= 0:
                        nc.tensor.matmul(
                            sp[:, q0 : q0 + 128],
                            ident,
                            tri,
                            start=False,
                            stop=True,
                            skip_group_check=True,
                        )
                    pt = ppool.tile([128, 512], BF16, tag="p")
                    nc.scalar.activation(
                        pt[:, q0:],
                        sp[:, q0:],
                        mybir.ActivationFunctionType.Exp,
                        scale=SCALE,
                    )
                    pts[tk] = (pt, q0)

                def stage2(tk):
                    pt, q0 = pts.pop(tk)
                    nc.tensor.matmul(
                        ot[:, q0:],
                        V[:, tk],
                        pt[:, q0:],
                        start=(tk == 0),
                        stop=(tk == ntk - 1),
                        skip_group_check=True,
                    )
                    for ch in range(q0 // 128, 4):
                        nc.tensor.matmul(
                            rsT[:, ch : ch + 1],
                            pt[:, 128 * ch : 128 * (ch + 1)],
                            ones_col,
                            start=(tk == 0 and ch == 0),
                            stop=(tk == ntk - 1 and ch == 3),
                            skip_group_check=True,
                        )

                # depth-2 software pipeline with fillers ahead of each OT
                last = b == BLOC - 1 and j == NBLK - 1
                for tk in range(ntk):
                    drain_until((b, tk // 4, 1))
                    stage1(tk)
                    if tk >= 2:
                        pop_filler(force=last)
                        stage2(tk - 2)
                pop_filler(force=last)
                stage2(ntk - 2)
                pop_filler(force=last)
                stage2(ntk - 1)

                # ---- block tail: normalize + transpose + store ----
                # per-quarter chains so the store DMAs overlap the next block
                recipT = small.tile([128, 4], F32, tag="recip")
                nc.vector.reciprocal(recipT, rsT)
                ot_sb = small.tile([128, 512], BF16, tag="otsb")
                o_sb = small.tile([128, 4, 128], F32, tag="osb")
                for t4 in range(4):
                    evac(
                        ot_sb[:, 128 * t4 : 128 * (t4 + 1)],
                        ot[:, 128 * t4 : 128 * (t4 + 1)],
                    )
                    pop_filler(250)
                    op = ps_t.tile([128, 1, 128], BF16, tag="tpose", name="op")
                    nc.tensor.transpose(
                        op[:, 0],
                        ot_sb[:, 128 * t4 : 128 * (t4 + 1)],
                        ident,
                    )
                    if t4 % 2 == 0:
                        nc.vector.tensor_scalar_mul(
                            o_sb[:, t4], op[:, 0], recipT[:, t4 : t4 + 1]
                        )
                    else:
                        nc.scalar.mul(o_sb[:, t4], op[:, 0], recipT[:, t4 : t4 + 1])
                    if last:
                        # final block: store each quarter as it completes so
                        # the closing DMA chain overlaps the remaining work
                        nc.sync.dma_start(
                            out=out_d[
                                b,
                                512 * j + 128 * t4 : 512 * j + 128 * (t4 + 1),
                                :,
                            ].rearrange("(t p) d -> p t d", p=128),
                            in_=o_sb[:, t4],
                        )
                if not last:
                    nc.sync.dma_start(
                        out=out_d[b, sl, :].rearrange("(t p) d -> p t d", p=128),
                        in_=o_sb,
                    )

        # drain any leftover fillers (shouldn't be any)
        while fillers:
            fillers.popleft()[1]()


_NC = None


def _get_nc():
    global _NC
    if _NC is None:
        _NC = _build()
    return _NC


def kernel(x, Wq, Wk, Wv):
    nc = _get_nc()
    x = np.ascontiguousarray(x, dtype=np.float32)
    in_maps = [
        {"x": x[BLOC * c : BLOC * (c + 1)], "Wq": Wq, "Wk": Wk, "Wv": Wv}
        for c in range(NCORES)
    ]
    res = run_bass_kernel_spmd(nc, in_maps, core_ids=list(range(NCORES)))
    return np.concatenate([res.results[c]["out"] for c in range(NCORES)], axis=0)


# revision 5
# speedup vs baseline: 1.4515x; 1.0130x over previous
"""Single-head causal attention (B=16, T=2048, C=1024, D=128) on 8 TRN2 cores.

Data-parallel over batch: each core handles 2 batches. All-bf16 compute
(measured rel err ~0.5% vs the 2e-2 gate; fp8 fails the max-err metric).

Per core, per batch b:
  X(b,g): x rows [512g..512g+512) f32 -> PE transposes (f32r bitcast data,
          bf16 identity: 1 cyc/row) -> evac-convert -> xT bf16 [C-part, T]
  P(b,j): qT/kT = W^T @ xT (512-wide moving), V = x @ Wv via xT-stationary
          chunks [T-part, D]
  A(b,j): per key tile (causally width-restricted to queries >= 128*iloc):
          S^T = kT_tile^T @ qT (PSUM), + triangular -1e30 mask via a PE
          matmul accumulate (ident^T @ tri) on the diagonal square,
          P^T = exp(scale*S^T) on ACT (bf16),
          OT += V_tile^T @ P^T,  rsT[:,ch] += pt_chunk^T @ ones (out [128,1]).
          Normalize via per-partition recip after the out-transpose.

Emission is a single stream with software pipelining: within an A block the
OT/rsT for tile i are emitted two tiles behind its scores, and X/P work for
later groups/batches is popped from a filler queue ahead of each OT so the
in-order PE queue never parks on an exp dependency.
"""

import numpy as np
from collections import deque

import concourse.bacc as bacc
import concourse.mybir as mybir
import concourse.tile as tile
from concourse.bass_utils import run_bass_kernel_spmd
from concourse.masks import make_identity

F32 = mybir.dt.float32
F32R = mybir.dt.float32r
BF16 = mybir.dt.bfloat16

B, T, C, D = 16, 2048, 1024, 128
NCORES = 8
BLOC = B // NCORES  # batches per core
NBLK = T // 512  # query blocks of width 512
NKT = T // 128  # key tiles of 128
SCALE = float(D) ** -0.5


def _build():
    nc = bacc.Bacc("TRN2", target_bir_lowering=False, debug=False, num_devices=NCORES)
    x_d = nc.dram_tensor("x", [BLOC, T, C], F32, kind="ExternalInput").ap()
    wq_d = nc.dram_tensor("Wq", [C, D], F32, kind="ExternalInput").ap()
    wk_d = nc.dram_tensor("Wk", [C, D], F32, kind="ExternalInput").ap()
    wv_d = nc.dram_tensor("Wv", [C, D], F32, kind="ExternalInput").ap()
    out_d = nc.dram_tensor("out", [BLOC, T, D], F32, kind="ExternalOutput").ap()

    with tile.TileContext(nc) as tc:
        _emit(nc, tc, x_d, (wq_d, wk_d, wv_d), out_d)
    nc.compile()
    return nc


def _emit(nc, tc, x_d, w_ds, out_d):
    from contextlib import ExitStack

    ctx = ExitStack()
    with ctx:
        const = ctx.enter_context(tc.tile_pool(name="const", bufs=1))
        xstage = ctx.enter_context(tc.tile_pool(name="xstage", bufs=6))
        xtp = ctx.enter_context(tc.tile_pool(name="xtp", bufs=1))
        qkv = ctx.enter_context(tc.tile_pool(name="qkv", bufs=2))
        ppool = ctx.enter_context(tc.tile_pool(name="ppool", bufs=8))
        small = ctx.enter_context(tc.tile_pool(name="small", bufs=4))
        # PSUM: 8 banks of [128, 2KB]
        ps_t = ctx.enter_context(tc.tile_pool(name="ps_t", bufs=2, space="PSUM"))
        ps_proj = ctx.enter_context(tc.tile_pool(name="ps_proj", bufs=2, space="PSUM"))
        ps_s = ctx.enter_context(tc.tile_pool(name="ps_s", bufs=2, space="PSUM"))
        ps_ot = ctx.enter_context(tc.tile_pool(name="ps_ot", bufs=1, space="PSUM"))
        ps_rs = ctx.enter_context(tc.tile_pool(name="ps_rs", bufs=1, space="PSUM"))

        # ---- x stage DMAs: one 512KB slab per 128 t-rows ----
        stages = {}

        def load_stage(b, s, split=False):
            st = xstage.tile([128, C], F32R, tag="stage", name="stage")
            if split:
                for h in range(2):
                    nc.sync.dma_start(
                        out=st[:, 512 * h : 512 * (h + 1)],
                        in_=x_d[
                            b, 128 * s : 128 * (s + 1), 512 * h : 512 * (h + 1)
                        ].bitcast(F32R),
                    )
            else:
                nc.sync.dma_start(
                    out=st, in_=x_d[b, 128 * s : 128 * (s + 1), :].bitcast(F32R)
                )
            stages[(b, s)] = st

        load_stage(0, 0, split=True)

        # W DMAs interleaved with the early slab loads: Wq before slab 1 so
        # the first q-projection isn't gated on weights
        w_sts = []
        for p in range(3):
            w_st = const.tile([128, 8, 128], F32, tag=f"wst{p}", name="wst")
            nc.sync.dma_start(
                out=w_st, in_=w_ds[p].rearrange("(k p) d -> p k d", p=128)
            )
            w_sts.append(w_st)
            load_stage(0, p + 1)

        # ---- constants ----
        ident_f = const.tile([128, 128], F32, tag="identf")
        make_identity(nc, ident_f)
        ident = const.tile([128, 128], BF16, tag="ident")
        nc.gpsimd.tensor_copy(ident, ident_f)
        ident_r = const.tile([128, 128], F32R, tag="identr")
        nc.gpsimd.tensor_copy(ident_r, ident_f)
        # triangular mask for the diagonal square: keep 0 iff q - k >= 0
        tri_f = const.tile([128, 128], F32, tag="trif")
        nc.gpsimd.memset(tri_f, 0.0)
        nc.gpsimd.affine_select(
            out=tri_f,
            in_=tri_f,
            compare_op=mybir.AluOpType.is_ge,
            fill=-1e30,
            base=0,
            pattern=[[1, 128]],
            channel_multiplier=-1,
        )
        tri = const.tile([128, 128], BF16, tag="tri")
        nc.gpsimd.tensor_copy(tri, tri_f)
        ones_col = const.tile([128, 1], BF16, tag="ones")
        nc.vector.memset(ones_col, 1.0)
        # weights: fp32 staging -> bf16 [128, 3, 8, 128]
        w_bf = const.tile([128, 3, 8, 128], BF16, tag="wbf")
        for p in range(3):
            nc.gpsimd.tensor_copy(w_bf[:, p], w_sts[p])

        # evac copies (PSUM -> SBUF) rotate over DVE and ACT
        # (GPSIMD/Pool cannot access PSUM at all).
        copy_fns = [
            lambda o, i: nc.vector.tensor_copy(o, i),
            lambda o, i: nc.scalar.copy(o, i),
        ]
        evac_state = [0]

        def evac(out_ap, in_ap, seq=(0, 1)):
            copy_fns[seq[evac_state[0] % len(seq)]](out_ap, in_ap)
            evac_state[0] += 1

        xTs = {}
        qTs, kTs, Vs = {}, {}, {}

        # ---------- emission units ----------
        def xgroup(b, s, half):
            """Transpose half of the [128 t, C] slab s: 4 c-chunks."""
            st = stages[(b, s)]
            xT = xTs[b]
            tp = ps_t.tile([128, 4, 128], F32R, tag="tpose")
            for i in range(4):
                cc = 4 * half + i
                nc.tensor.transpose(
                    tp[:, i],
                    st[:, 128 * cc : 128 * (cc + 1)],
                    ident_r,
                )
            evac(
                xT[:, 4 * half : 4 * half + 4, 128 * s : 128 * (s + 1)],
                tp,
            )

        proj_accs = {}

        def proj_qk_slab(b, j, p, s4):
            """qT/kT for one 128-col t-slice: 8 kk matmuls, 128-wide."""
            xT = xTs[b]
            dst = (qTs if p == 0 else kTs)[b]
            if s4 == 0:
                proj_accs[(b, j, p)] = ps_proj.tile(
                    [128, 512], F32, tag="proj", name="proj"
                )
            acc = proj_accs[(b, j, p)]
            tsl = slice(512 * j + 128 * s4, 512 * j + 128 * (s4 + 1))
            for kk in range(8):
                nc.tensor.matmul(
                    acc[:, 128 * s4 : 128 * (s4 + 1)],
                    w_bf[:, p, kk],
                    xT[:, kk, tsl],
                    start=(s4 == 0 and kk == 0),
                    stop=(s4 == 3 and kk == 7),
                    skip_group_check=True,
                )
            if s4 == 3:
                evac(dst[:, 512 * j : 512 * (j + 1)], acc)
                del proj_accs[(b, j, p)]

        def proj_v_sub(b, j, tc4):
            """One 128-row t-chunk of V (8 kk matmuls); tc4 3 evacuates."""
            xT = xTs[b]
            V = Vs[b]
            if tc4 == 0:
                proj_accs[(b, j, 2)] = ps_proj.tile(
                    [128, 512], F32, tag="proj", name="vacc"
                )
            vacc = proj_accs[(b, j, 2)]
            tsl = slice(512 * j + 128 * tc4, 512 * j + 128 * (tc4 + 1))
            for kk in range(8):
                nc.tensor.matmul(
                    vacc[:, 128 * tc4 : 128 * (tc4 + 1)],
                    xT[:, kk, tsl],
                    w_bf[:, 2, kk],
                    start=(tc4 == 0 and kk == 0),
                    stop=(tc4 == 3 and kk == 7),
                    skip_group_check=True,
                )
            if tc4 == 3:
                evac(
                    V[:, 4 * j : 4 * (j + 1)],
                    vacc.rearrange("p (m d) -> p m d", m=4),
                )
                del proj_accs[(b, j, 2)]

        # filler queue: list of (marker, closure). Marker (b, j) means this
        # unit belongs to X/P of block j of batch b; A(b, j) requires all
        # markers <= (b, j) emitted.
        # filler unit = (marker, est_pe_ns, closure); marker = (b, g, kind):
        # kind 0 units must precede A(b, g) block start (xT slabs + qT);
        # kind 1 units (kT/V of block g) are only needed by tiles >= 4g.
        fillers = deque()
        slabs = [(b, s) for b in range(BLOC) for s in range(4 * NBLK)]
        for b in range(BLOC):
            for g in range(NBLK):
                mk0 = (b, g, 0)
                mk1 = (b, g, 1)
                for s4 in range(4):
                    s = 4 * g + s4
                    # prefetch the slab load 4 slabs ahead of its transposes
                    idx = slabs.index((b, s)) + 4
                    if idx < len(slabs):
                        pb, ps2 = slabs[idx]
                        fillers.append(
                            (mk0, 0, lambda pb=pb, ps2=ps2: load_stage(pb, ps2))
                        )
                    for half in range(2):
                        fillers.append(
                            (mk0, 230, lambda b=b, s=s, h=half: xgroup(b, s, h))
                        )
                    # q and v lag one slab behind the transposes so the
                    # xT evacuations are complete when they issue
                    # (2 live proj accumulators: q + v; k comes after)
                    if s4 >= 1:
                        fillers.append(
                            (mk0, 460,
                             lambda b=b, g=g, s=s4 - 1: proj_qk_slab(b, g, 0, s))
                        )
                        fillers.append(
                            (mk1, 460, lambda b=b, g=g, s=s4 - 1: proj_v_sub(b, g, s))
                        )
                fillers.append(
                    (mk0, 460, lambda b=b, g=g: proj_qk_slab(b, g, 0, 3))
                )
                fillers.append((mk1, 460, lambda b=b, g=g: proj_v_sub(b, g, 3)))
                for s4 in range(4):
                    fillers.append(
                        (mk1, 460, lambda b=b, g=g, s=s4: proj_qk_slab(b, g, 1, s))
                    )

        def drain_until(marker):
            # the queue is not strictly marker-sorted (kind-1 units are
            # interleaved), so filter the whole queue
            rest = []
            while fillers:
                mk, est, fn = fillers.popleft()
                if mk <= marker:
                    fn()
                else:
                    rest.append((mk, est, fn))
            fillers.extend(rest)

        def pop_filler(budget_ns=350, force=False):
            spent = 0
            while fillers and spent < budget_ns:
                if not force and len(fillers) <= 8:
                    return
                _, est, fn = fillers.popleft()
                fn()
                spent += est if est else 0

        for b in range(BLOC):
            xTs[b] = xtp.tile([128, 8, T], BF16, tag="xT", name="xT")
            qTs[b] = qkv.tile([128, T], BF16, tag="qT", name="qT")
            kTs[b] = qkv.tile([128, T], BF16, tag="kT", name="kT")
            Vs[b] = qkv.tile([128, NKT, 128], BF16, tag="V", name="V")

        # ---------- main schedule ----------
        drain_until((0, 0, 1))  # startup: PE is DMA-starved, emit eagerly
        for b in range(BLOC):
            for j in range(NBLK):
                drain_until((b, j, 0))
                qT, kT, V = qTs[b], kTs[b], Vs[b]
                sl = slice(512 * j, 512 * (j + 1))
                ntk = 4 * (j + 1)
                ot = ps_ot.tile([128, 512], F32, tag="ot")
                rsT = ps_rs.tile([128, 4], F32, tag="rs")
                pts = {}

                def stage1(tk):
                    iloc = tk - 4 * j
                    q0 = 128 * iloc if iloc >= 0 else 0
                    sp = ps_s.tile([128, 512], F32, tag="s")
                    nc.tensor.matmul(
                        sp[:, q0:],
                        kT[:, 128 * tk : 128 * (tk + 1)],
                        qT[:, 512 * j + q0 : 512 * (j + 1)],
                        start=True,
                        stop=(iloc < 0),
                        skip_group_check=True,
                    )
                    if iloc >= 0:
                        nc.tensor.matmul(
                            sp[:, q0 : q0 + 128],
                            ident,
                            tri,
                            start=False,
                            stop=True,
                            skip_group_check=True,
                        )
                    pt = ppool.tile([128, 512], BF16, tag="p")
                    nc.scalar.activation(
                        pt[:, q0:],
                        sp[:, q0:],
                        mybir.ActivationFunctionType.Exp,
                        scale=SCALE,
                    )
                    pts[tk] = (pt, q0)

                def stage2(tk):
                    pt, q0 = pts.pop(tk)
                    nc.tensor.matmul(
                        ot[:, q0:],
                        V[:, tk],
                        pt[:, q0:],
                        start=(tk == 0),
                        stop=(tk == ntk - 1),
                        skip_group_check=True,
                    )
                    for ch in range(q0 // 128, 4):
                        nc.tensor.matmul(
                            rsT[:, ch : ch + 1],
                            pt[:, 128 * ch : 128 * (ch + 1)],
                            ones_col,
                            start=(tk == 0 and ch == 0),
                            stop=(tk == ntk - 1 and ch == 3),
                            skip_group_check=True,
                        )

                last = b == BLOC - 1 and j == NBLK - 1
                recipT = small.tile([128, 4], F32, tag="recip")
                ot_sb = small.tile([128, 512], BF16, tag="otsb")
                o_sb = small.tile([128, 4, 128], F32, tag="osb")

                def tail_a(ch):
                    nc.vector.reciprocal(
                        recipT[:, ch : ch + 1], rsT[:, ch : ch + 1]
                    )
                    evac(
                        ot_sb[:, 128 * ch : 128 * (ch + 1)],
                        ot[:, 128 * ch : 128 * (ch + 1)],
                    )

                def tail_b(ch):
                    op = ps_t.tile([128, 1, 128], BF16, tag="tpose", name="op")
                    nc.tensor.transpose(
                        op[:, 0], ot_sb[:, 128 * ch : 128 * (ch + 1)], ident
                    )
                    if ch % 2 == 0:
                        nc.vector.tensor_scalar_mul(
                            o_sb[:, ch], op[:, 0], recipT[:, ch : ch + 1]
                        )
                    else:
                        nc.scalar.mul(o_sb[:, ch], op[:, 0], recipT[:, ch : ch + 1])
                    if last:
                        nc.sync.dma_start(
                            out=out_d[
                                b,
                                512 * j + 128 * ch : 512 * j + 128 * (ch + 1),
                                :,
                            ].rearrange("(t p) d -> p t d", p=128),
                            in_=o_sb[:, ch],
                        )

                # depth-2 software pipeline with fillers ahead of each OT.
                # In the final block the tail is pipelined into the last
                # diagonal tiles (quarter ch of OT/rsT is final after tile
                # 4j+ch); elsewhere the tail runs after the loop, hidden by
                # filler pops.
                for tk in range(ntk):
                    drain_until((b, tk // 4, 1))
                    stage1(tk)
                    if tk >= 2:
                        pop_filler(force=last)
                        stage2(tk - 2)
                        if last:
                            i = tk - 2
                            if i > 4 * j:
                                tail_b(i - 4 * j - 1)
                            if i >= 4 * j:
                                tail_a(i - 4 * j)
                pop_filler(force=last)
                stage2(ntk - 2)
                if last:
                    tail_b(1)
                    tail_a(2)
                pop_filler(force=last)
                stage2(ntk - 1)
                if last:
                    tail_b(2)
                    tail_a(3)
                    tail_b(3)
                else:
                    for t4 in range(4):
                        tail_a(t4)
                        pop_filler(250)
                        tail_b(t4)
                    nc.sync.dma_start(
                        out=out_d[b, sl, :].rearrange("(t p) d -> p t d", p=128),
                        in_=o_sb,
                    )

        # drain any leftover fillers (shouldn't be any)
        while fillers:
            fillers.popleft()[1]()


_NC = None


def _get_nc():
    global _NC
    if _NC is None:
        _NC = _build()
    return _NC


def kernel(x, Wq, Wk, Wv):
    nc = _get_nc()
    x = np.ascontiguousarray(x, dtype=np.float32)
    in_maps = [
        {"x": x[BLOC * c : BLOC * (c + 1)], "Wq": Wq, "Wk": Wk, "Wv": Wv}
        for c in range(NCORES)
    ]
    res = run_bass_kernel_spmd(nc, in_maps, core_ids=list(range(NCORES)))
    return np.concatenate([res.results[c]["out"] for c in range(NCORES)], axis=0)


# revision 6
# speedup vs baseline: 1.5138x; 1.0429x over previous
"""Single-head causal attention (B=16, T=2048, C=1024, D=128) on 8 TRN2 cores.

Data-parallel over batch: each core handles 2 batches. All-bf16 compute
(measured rel err ~0.5% vs the 2e-2 gate; fp8 fails the max-err metric).

Per core, per batch b:
  X(b,g): x rows [512g..512g+512) f32 -> PE transposes (f32r bitcast data,
          bf16 identity: 1 cyc/row) -> evac-convert -> xT bf16 [C-part, T]
  P(b,j): qT/kT = W^T @ xT (512-wide moving), V = x @ Wv via xT-stationary
          chunks [T-part, D]
  A(b,j): per key tile (causally width-restricted to queries >= 128*iloc):
          S^T = kT_tile^T @ qT (PSUM), + triangular -1e30 mask via a PE
          matmul accumulate (ident^T @ tri) on the diagonal square,
          P^T = exp(scale*S^T) on ACT (bf16),
          OT += V_tile^T @ P^T,  rsT[:,ch] += pt_chunk^T @ ones (out [128,1]).
          Normalize via per-partition recip after the out-transpose.

Emission is a single stream with software pipelining: within an A block the
OT/rsT for tile i are emitted two tiles behind its scores, and X/P work for
later groups/batches is popped from a filler queue ahead of each OT so the
in-order PE queue never parks on an exp dependency.
"""

import numpy as np
from collections import deque

import concourse.bacc as bacc
import concourse.mybir as mybir
import concourse.tile as tile
from concourse.bass_utils import run_bass_kernel_spmd
from concourse.masks import make_identity

F32 = mybir.dt.float32
F32R = mybir.dt.float32r
BF16 = mybir.dt.bfloat16

B, T, C, D = 16, 2048, 1024, 128
NCORES = 8
BLOC = B // NCORES  # batches per core
NBLK = T // 512  # query blocks of width 512
NKT = T // 128  # key tiles of 128
SCALE = float(D) ** -0.5


def _build():
    nc = bacc.Bacc("TRN2", target_bir_lowering=False, debug=False, num_devices=NCORES)
    x_d = nc.dram_tensor("x", [BLOC, T, C], F32, kind="ExternalInput").ap()
    wq_d = nc.dram_tensor("Wq", [C, D], F32, kind="ExternalInput").ap()
    wk_d = nc.dram_tensor("Wk", [C, D], F32, kind="ExternalInput").ap()
    wv_d = nc.dram_tensor("Wv", [C, D], F32, kind="ExternalInput").ap()
    out_d = nc.dram_tensor("out", [BLOC, T, D], F32, kind="ExternalOutput").ap()

    with tile.TileContext(nc) as tc:
        _emit(nc, tc, x_d, (wq_d, wk_d, wv_d), out_d)
    nc.compile()
    return nc


def _emit(nc, tc, x_d, w_ds, out_d):
    from contextlib import ExitStack

    ctx = ExitStack()
    with ctx:
        const = ctx.enter_context(tc.tile_pool(name="const", bufs=1))
        xstage = ctx.enter_context(tc.tile_pool(name="xstage", bufs=6))
        xtp = ctx.enter_context(tc.tile_pool(name="xtp", bufs=1))
        qkv = ctx.enter_context(tc.tile_pool(name="qkv", bufs=2))
        ppool = ctx.enter_context(tc.tile_pool(name="ppool", bufs=8))
        small = ctx.enter_context(tc.tile_pool(name="small", bufs=4))
        # PSUM: 8 banks of [128, 2KB]
        ps_t = ctx.enter_context(tc.tile_pool(name="ps_t", bufs=2, space="PSUM"))
        ps_proj = ctx.enter_context(tc.tile_pool(name="ps_proj", bufs=2, space="PSUM"))
        ps_s = ctx.enter_context(tc.tile_pool(name="ps_s", bufs=2, space="PSUM"))
        ps_ot = ctx.enter_context(tc.tile_pool(name="ps_ot", bufs=1, space="PSUM"))
        ps_rs = ctx.enter_context(tc.tile_pool(name="ps_rs", bufs=1, space="PSUM"))

        # ---- x stage DMAs: one 512KB slab per 128 t-rows ----
        stages = {}

        def load_stage(b, s, split=False):
            st = xstage.tile([128, C], F32R, tag="stage", name="stage")
            if split:
                for h in range(2):
                    nc.sync.dma_start(
                        out=st[:, 512 * h : 512 * (h + 1)],
                        in_=x_d[
                            b, 128 * s : 128 * (s + 1), 512 * h : 512 * (h + 1)
                        ].bitcast(F32R),
                    )
            else:
                nc.sync.dma_start(
                    out=st, in_=x_d[b, 128 * s : 128 * (s + 1), :].bitcast(F32R)
                )
            stages[(b, s)] = st

        load_stage(0, 0, split=True)

        # W DMAs interleaved with the early slab loads: Wq before slab 1 so
        # the first q-projection isn't gated on weights
        w_sts = []
        for p in range(3):
            w_st = const.tile([128, 8, 128], F32, tag=f"wst{p}", name="wst")
            nc.sync.dma_start(
                out=w_st, in_=w_ds[p].rearrange("(k p) d -> p k d", p=128)
            )
            w_sts.append(w_st)
            load_stage(0, p + 1)

        # ---- constants ----
        ident_f = const.tile([128, 128], F32, tag="identf")
        make_identity(nc, ident_f)
        ident = const.tile([128, 128], BF16, tag="ident")
        nc.gpsimd.tensor_copy(ident, ident_f)
        ident_r = const.tile([128, 128], F32R, tag="identr")
        nc.gpsimd.tensor_copy(ident_r, ident_f)
        # triangular mask for the diagonal square: keep 0 iff q - k >= 0
        tri_f = const.tile([128, 128], F32, tag="trif")
        nc.gpsimd.memset(tri_f, 0.0)
        nc.gpsimd.affine_select(
            out=tri_f,
            in_=tri_f,
            compare_op=mybir.AluOpType.is_ge,
            fill=-1e30,
            base=0,
            pattern=[[1, 128]],
            channel_multiplier=-1,
        )
        tri = const.tile([128, 128], BF16, tag="tri")
        nc.gpsimd.tensor_copy(tri, tri_f)
        ones_col = const.tile([128, 1], BF16, tag="ones")
        nc.vector.memset(ones_col, 1.0)
        # weights: fp32 staging -> bf16, prescaled by 1+2^-9 to cancel the
        # truncate-to-bf16 bias on x (see xgroup)
        w_bf = const.tile([128, 3, 8, 128], BF16, tag="wbf")
        for p in range(3):
            nc.gpsimd.tensor_scalar_mul(w_bf[:, p], w_sts[p], 1.001953125)

        # evac copies (PSUM -> SBUF) rotate over DVE and ACT
        # (GPSIMD/Pool cannot access PSUM at all).
        copy_fns = [
            lambda o, i: nc.vector.tensor_copy(o, i),
            lambda o, i: nc.scalar.copy(o, i),
        ]
        evac_state = [0]

        def evac(out_ap, in_ap, seq=(0, 1)):
            copy_fns[seq[evac_state[0] % len(seq)]](out_ap, in_ap)
            evac_state[0] += 1

        xTs = {}
        qTs, kTs, Vs = {}, {}, {}

        # ---------- emission units ----------
        def xgroup(b, s, half):
            """Transpose half of the [128 t, C] slab s: 4 c-chunks.

            Only the high 16 bits of each little-endian fp32 are moved: that
            bit pattern IS truncate-to-bf16(x), so the transpose runs as an
            all-bf16 matmul (1 cyc/row vs f32r's 1.5). The truncation bias
            (x scaled by ~1-2^-9) is cancelled by the weight prescale.
            """
            st_hi = stages[(b, s)].bitcast(BF16).rearrange(
                "p (c two) -> p c two", two=2
            )
            xT = xTs[b]
            tp = ps_t.tile([128, 4, 128], BF16, tag="tpose")
            for i in range(4):
                cc = 4 * half + i
                nc.tensor.transpose(
                    tp[:, i],
                    st_hi[:, 128 * cc : 128 * (cc + 1), 1],
                    ident,
                )
            evac(
                xT[:, 4 * half : 4 * half + 4, 128 * s : 128 * (s + 1)],
                tp,
            )

        proj_accs = {}

        def proj_qk_slab(b, j, p, s4):
            """qT/kT for one 128-col t-slice: 8 kk matmuls, 128-wide."""
            xT = xTs[b]
            dst = (qTs if p == 0 else kTs)[b]
            if s4 == 0:
                proj_accs[(b, j, p)] = ps_proj.tile(
                    [128, 512], F32, tag="proj", name="proj"
                )
            acc = proj_accs[(b, j, p)]
            tsl = slice(512 * j + 128 * s4, 512 * j + 128 * (s4 + 1))
            for kk in range(8):
                nc.tensor.matmul(
                    acc[:, 128 * s4 : 128 * (s4 + 1)],
                    w_bf[:, p, kk],
                    xT[:, kk, tsl],
                    start=(s4 == 0 and kk == 0),
                    stop=(s4 == 3 and kk == 7),
                    skip_group_check=True,
                )
            if s4 == 3:
                evac(dst[:, 512 * j : 512 * (j + 1)], acc)
                del proj_accs[(b, j, p)]

        def proj_v_sub(b, j, tc4):
            """One 128-row t-chunk of V (8 kk matmuls); tc4 3 evacuates."""
            xT = xTs[b]
            V = Vs[b]
            if tc4 == 0:
                proj_accs[(b, j, 2)] = ps_proj.tile(
                    [128, 512], F32, tag="proj", name="vacc"
                )
            vacc = proj_accs[(b, j, 2)]
            tsl = slice(512 * j + 128 * tc4, 512 * j + 128 * (tc4 + 1))
            for kk in range(8):
                nc.tensor.matmul(
                    vacc[:, 128 * tc4 : 128 * (tc4 + 1)],
                    xT[:, kk, tsl],
                    w_bf[:, 2, kk],
                    start=(tc4 == 0 and kk == 0),
                    stop=(tc4 == 3 and kk == 7),
                    skip_group_check=True,
                )
            if tc4 == 3:
                evac(
                    V[:, 4 * j : 4 * (j + 1)],
                    vacc.rearrange("p (m d) -> p m d", m=4),
                )
                del proj_accs[(b, j, 2)]

        # filler queue: list of (marker, closure). Marker (b, j) means this
        # unit belongs to X/P of block j of batch b; A(b, j) requires all
        # markers <= (b, j) emitted.
        # filler unit = (marker, est_pe_ns, closure); marker = (b, g, kind):
        # kind 0 units must precede A(b, g) block start (xT slabs + qT);
        # kind 1 units (kT/V of block g) are only needed by tiles >= 4g.
        fillers = deque()
        slabs = [(b, s) for b in range(BLOC) for s in range(4 * NBLK)]
        for b in range(BLOC):
            for g in range(NBLK):
                mk0 = (b, g, 0)
                mk1 = (b, g, 1)
                for s4 in range(4):
                    s = 4 * g + s4
                    # prefetch the slab load 4 slabs ahead of its transposes
                    idx = slabs.index((b, s)) + 4
                    if idx < len(slabs):
                        pb, ps2 = slabs[idx]
                        fillers.append(
                            (mk0, 0, lambda pb=pb, ps2=ps2: load_stage(pb, ps2))
                        )
                    for half in range(2):
                        fillers.append(
                            (mk0, 230, lambda b=b, s=s, h=half: xgroup(b, s, h))
                        )
                    # q and v lag one slab behind the transposes so the
                    # xT evacuations are complete when they issue
                    # (2 live proj accumulators: q + v; k comes after)
                    if s4 >= 1:
                        fillers.append(
                            (mk0, 460,
                             lambda b=b, g=g, s=s4 - 1: proj_qk_slab(b, g, 0, s))
                        )
                        fillers.append(
                            (mk1, 460, lambda b=b, g=g, s=s4 - 1: proj_v_sub(b, g, s))
                        )
                fillers.append(
                    (mk0, 460, lambda b=b, g=g: proj_qk_slab(b, g, 0, 3))
                )
                fillers.append((mk1, 460, lambda b=b, g=g: proj_v_sub(b, g, 3)))
                for s4 in range(4):
                    fillers.append(
                        (mk1, 460, lambda b=b, g=g, s=s4: proj_qk_slab(b, g, 1, s))
                    )

        def drain_until(marker):
            # the queue is not strictly marker-sorted (kind-1 units are
            # interleaved), so filter the whole queue
            rest = []
            while fillers:
                mk, est, fn = fillers.popleft()
                if mk <= marker:
                    fn()
                else:
                    rest.append((mk, est, fn))
            fillers.extend(rest)

        def pop_filler(budget_ns=350, force=False):
            spent = 0
            while fillers and spent < budget_ns:
                if not force and len(fillers) <= 8:
                    return
                _, est, fn = fillers.popleft()
                fn()
                spent += est if est else 0

        for b in range(BLOC):
            xTs[b] = xtp.tile([128, 8, T], BF16, tag="xT", name="xT")
            qTs[b] = qkv.tile([128, T], BF16, tag="qT", name="qT")
            kTs[b] = qkv.tile([128, T], BF16, tag="kT", name="kT")
            Vs[b] = qkv.tile([128, NKT, 128], BF16, tag="V", name="V")

        # ---------- main schedule ----------
        drain_until((0, 0, 1))  # startup: PE is DMA-starved, emit eagerly
        for b in range(BLOC):
            for j in range(NBLK):
                drain_until((b, j, 0))
                qT, kT, V = qTs[b], kTs[b], Vs[b]
                sl = slice(512 * j, 512 * (j + 1))
                ntk = 4 * (j + 1)
                ot = ps_ot.tile([128, 512], F32, tag="ot")
                rsT = ps_rs.tile([128, 4], F32, tag="rs")
                pts = {}

                def stage1(tk):
                    iloc = tk - 4 * j
                    q0 = 128 * iloc if iloc >= 0 else 0
                    sp = ps_s.tile([128, 512], F32, tag="s")
                    nc.tensor.matmul(
                        sp[:, q0:],
                        kT[:, 128 * tk : 128 * (tk + 1)],
                        qT[:, 512 * j + q0 : 512 * (j + 1)],
                        start=True,
                        stop=(iloc < 0),
                        skip_group_check=True,
                    )
                    if iloc >= 0:
                        nc.tensor.matmul(
                            sp[:, q0 : q0 + 128],
                            ident,
                            tri,
                            start=False,
                            stop=True,
                            skip_group_check=True,
                        )
                    pt = ppool.tile([128, 512], BF16, tag="p")
                    nc.scalar.activation(
                        pt[:, q0:],
                        sp[:, q0:],
                        mybir.ActivationFunctionType.Exp,
                        scale=SCALE,
                    )
                    pts[tk] = (pt, q0)

                def stage2(tk):
                    pt, q0 = pts.pop(tk)
                    nc.tensor.matmul(
                        ot[:, q0:],
                        V[:, tk],
                        pt[:, q0:],
                        start=(tk == 0),
                        stop=(tk == ntk - 1),
                        skip_group_check=True,
                    )
                    for ch in range(q0 // 128, 4):
                        nc.tensor.matmul(
                            rsT[:, ch : ch + 1],
                            pt[:, 128 * ch : 128 * (ch + 1)],
                            ones_col,
                            start=(tk == 0 and ch == 0),
                            stop=(tk == ntk - 1 and ch == 3),
                            skip_group_check=True,
                        )

                last = b == BLOC - 1 and j == NBLK - 1
                recipT = small.tile([128, 4], F32, tag="recip")
                ot_sb = small.tile([128, 512], BF16, tag="otsb")
                o_sb = small.tile([128, 4, 128], F32, tag="osb")

                def tail_a(ch):
                    nc.vector.reciprocal(
                        recipT[:, ch : ch + 1], rsT[:, ch : ch + 1]
                    )
                    evac(
                        ot_sb[:, 128 * ch : 128 * (ch + 1)],
                        ot[:, 128 * ch : 128 * (ch + 1)],
                    )

                def tail_b(ch):
                    op = ps_t.tile([128, 1, 128], BF16, tag="tpose", name="op")
                    nc.tensor.transpose(
                        op[:, 0], ot_sb[:, 128 * ch : 128 * (ch + 1)], ident
                    )
                    if ch % 2 == 0:
                        nc.vector.tensor_scalar_mul(
                            o_sb[:, ch], op[:, 0], recipT[:, ch : ch + 1]
                        )
                    else:
                        nc.scalar.mul(o_sb[:, ch], op[:, 0], recipT[:, ch : ch + 1])
                    if last:
                        nc.sync.dma_start(
                            out=out_d[
                                b,
                                512 * j + 128 * ch : 512 * j + 128 * (ch + 1),
                                :,
                            ].rearrange("(t p) d -> p t d", p=128),
                            in_=o_sb[:, ch],
                        )

                # depth-2 software pipeline with fillers ahead of each OT.
                # In the final block the tail is pipelined into the last
                # diagonal tiles (quarter ch of OT/rsT is final after tile
                # 4j+ch); elsewhere the tail runs after the loop, hidden by
                # filler pops.
                for tk in range(ntk):
                    drain_until((b, tk // 4, 1))
                    stage1(tk)
                    if tk >= 2:
                        pop_filler(force=last)
                        stage2(tk - 2)
                        if last:
                            i = tk - 2
                            if i > 4 * j:
                                tail_b(i - 4 * j - 1)
                            if i >= 4 * j:
                                tail_a(i - 4 * j)
                pop_filler(force=last)
                stage2(ntk - 2)
                if last:
                    tail_b(1)
                    tail_a(2)
                pop_filler(force=last)
                stage2(ntk - 1)
                if last:
                    tail_b(2)
                    tail_a(3)
                    tail_b(3)
                else:
                    for t4 in range(4):
                        tail_a(t4)
                        pop_filler(250)
                        tail_b(t4)
                    nc.sync.dma_start(
                        out=out_d[b, sl, :].rearrange("(t p) d -> p t d", p=128),
                        in_=o_sb,
                    )

        # drain any leftover fillers (shouldn't be any)
        while fillers:
            fillers.popleft()[1]()


_NC = None


def _get_nc():
    global _NC
    if _NC is None:
        _NC = _build()
    return _NC


def kernel(x, Wq, Wk, Wv):
    nc = _get_nc()
    x = np.ascontiguousarray(x, dtype=np.float32)
    in_maps = [
        {"x": x[BLOC * c : BLOC * (c + 1)], "Wq": Wq, "Wk": Wk, "Wv": Wv}
        for c in range(NCORES)
    ]
    res = run_bass_kernel_spmd(nc, in_maps, core_ids=list(range(NCORES)))
    return np.concatenate([res.results[c]["out"] for c in range(NCORES)], axis=0)
